# revision 1
# baseline (speedup 1.0000x reference)
"""DeepSeek-V3 MLA attention kernel for 8 Trainium2 NeuronCores.

Problem: nn_DeepSeekV3_1Attention (B=2, S=2048, D=2048, H=16, NOPE=128,
ROPE=64, VD=128, QL=KVL=512), fp32 reference, causal.

Sharding: data-parallel over batch (2 groups of 4 cores) x tensor-parallel
over heads (4 heads per core). Each core computes its batch's shared
projections (c_q, c_kv, k_rope) redundantly, runs MLA attention for its 4
heads, and produces a partial out-projection (its heads' rows of out_w).
Host sums the 4 partials per batch.

All large tensors live on-chip in "transposed" layout (sequence on the
free dimension) so every matmul contracts over the partition dim without
any on-device transposes of activations:
  scores^T[k, q] = (c_kv^T chunk).T @ q_pe^T  (+ rope term)
  softmax is computed unnormalized (exp without max subtraction - scores
  are O(3) so exp is safe), with row sums via a ones-vector matmul, and
  normalization deferred past the (linear) PV and value-up projections.

Matmuls use float32r (tf32-like, 1 cycle/row at N>=512) for the Q/K path
and bf16 for the attention-value / output path.
"""

import numpy as np
import ml_dtypes

from concourse import bacc
import concourse.bass as bass
import concourse.mybir as mybir
import concourse.tile as tile
from concourse.bass_utils import run_bass_kernel_spmd
from concourse.masks import make_identity

F32 = mybir.dt.float32
F32R = mybir.dt.float32r
BF16 = mybir.dt.bfloat16
AF = mybir.ActivationFunctionType

B, S, D = 2, 2048, 2048
H = 16
NOPE, ROPE, VD = 128, 64, 128
QL, KVL = 512, 512
HPC = 4    # heads per core
G = 4      # cores per batch group
SCALE = float(1.0 / np.sqrt(np.float32(NOPE + ROPE)))

ROPE_WAVELENGTH = 10000.0
ROPE_SCALE = 40.0
BETA_FAST, BETA_SLOW = 32.0, 1.0
OLD_CTX = 4096.0
MSCALE = 1.0
PI = 3.14159265358979

NDC = D // 128          # 16 d-chunks
NQLC = QL // 128        # 4 ql chunks
NKVC = KVL // 128       # 4 kv chunks
NKC = S // 128          # 16 key chunks
NQB = S // 512          # 4 query blocks
NSB = S // 256          # 8 s-blocks (phase 1)


def _rope_tables():
    j = np.arange(0, ROPE, 2, dtype=np.float32) / ROPE
    freqs = (1.0 / (ROPE_WAVELENGTH ** j)).astype(np.float32)
    wavelengths = 2.0 * PI / freqs
    ramp = np.clip((wavelengths / OLD_CTX - BETA_SLOW) / (BETA_FAST - BETA_SLOW),
                   0.0, 1.0)
    scale = (1.0 - ramp) + ramp * ROPE_SCALE
    inv_freq = freqs / scale
    t = np.arange(S, dtype=np.float32)
    fr = t[:, None] * inv_freq[None, :]
    cos = (np.cos(fr) * MSCALE).astype(np.float32).T        # [32, S]
    sin = (np.sin(fr) * MSCALE).astype(np.float32).T
    cosT = np.ascontiguousarray(np.concatenate([cos, cos], 0))    # [64, S]
    sinT = np.ascontiguousarray(np.concatenate([-sin, sin], 0))   # [64, S]
    return cosT, sinT


def _masks():
    # multiplicative 0/1 masks applied to exp(scores) on the diagonal chunks
    k = np.arange(128)[:, None]
    q = np.arange(512)[None, :]
    ms = []
    for m in range(4):
        allow = (k + m * 128) <= q
        ms.append(np.where(allow, np.float32(1.0), np.float32(0.0)))
    return np.ascontiguousarray(np.stack(ms, axis=1))    # [128, 4, 512]


def _emit_rope(nc, pool, out_ap, raw_ap, cos_ap, sin_ap):
    """out(F32R) = raw*cos + swap(raw)*sin  (rows 0:32 <-> 32:64 swapped)."""
    n = raw_ap.shape[-1]
    sw = pool.tile([ROPE, n], F32, tag="rope_swap")
    nc.vector.tensor_copy(sw[0:32, :], raw_ap[32:64, :])
    nc.vector.tensor_copy(sw[32:64, :], raw_ap[0:32, :])
    nc.vector.tensor_mul(raw_ap, raw_ap, cos_ap)      # in place
    nc.vector.tensor_mul(sw[:, :], sw[:, :], sin_ap)
    nc.vector.tensor_add(out_ap, raw_ap, sw[:, :])    # writes f32r (rounds)


def build_nc():
    nc = bacc.Bacc("TRN2", target_bir_lowering=False, debug=False,
                   enable_asserts=False, num_devices=8)

    hsT = nc.dram_tensor("hsT", [D, S], F32R, kind="ExternalInput").ap()
    qdw = nc.dram_tensor("qdw", [D, QL], F32R, kind="ExternalInput").ap()
    kvdw = nc.dram_tensor("kvdw", [D, KVL], F32R, kind="ExternalInput").ap()
    krw = nc.dram_tensor("krw", [D, ROPE], F32R, kind="ExternalInput").ap()
    qnw = nc.dram_tensor("qnw", [QL, HPC * NOPE], F32R, kind="ExternalInput").ap()
    qrw = nc.dram_tensor("qrw", [QL, HPC * ROPE], F32R, kind="ExternalInput").ap()
    wukT = nc.dram_tensor("wukT", [HPC * KVL, NOPE], F32R, kind="ExternalInput").ap()
    wuv4 = nc.dram_tensor("wuv4", [KVL, HPC * VD], F32R, kind="ExternalInput").ap()
    owg = nc.dram_tensor("owg", [HPC * VD, D], BF16, kind="ExternalInput").ap()
    cosd = nc.dram_tensor("cosd", [ROPE, S], BF16, kind="ExternalInput").ap()
    sind = nc.dram_tensor("sind", [ROPE, S], BF16, kind="ExternalInput").ap()
    maskd = nc.dram_tensor("maskd", [128, 4, 512], BF16, kind="ExternalInput").ap()
    outT = nc.dram_tensor("outT", [D, S], F32, kind="ExternalOutput").ap()

    hsT_r = hsT.rearrange("(c p) s -> p c s", p=128)      # [128, 16, S]
    qdw_r = qdw.rearrange("(c p) q -> p c q", p=128)      # [128, 16, 512]
    kvdw_r = kvdw.rearrange("(c p) q -> p c q", p=128)
    krw_r = krw.rearrange("(c p) q -> p c q", p=128)      # [128, 16, 64]
    qnw_r = qnw.rearrange("(c p) n -> p c n", p=128)      # [128, 4, 512]
    qrw_r = qrw.rearrange("(c p) n -> p c n", p=128)      # [128, 4, 256]
    wukT_r = wukT.rearrange("(c p) n -> p c n", p=128)    # [128, 16, 128]
    wuv4_r = wuv4.rearrange("(c p) v -> p c v", p=128)    # [128, 4, 512]
    owg_r = owg.rearrange("(h p) d -> p h d", p=128)      # [128, 4, D]

    with tile.TileContext(nc) as tc:
        with tc.tile_pool(name="A", bufs=1) as A:
            c_qT = A.tile([128, NQLC, S], F32R, tag="c_qT")
            c_kvT = A.tile([128, NQLC, S], F32R, tag="c_kvT")
            k_ropeT = A.tile([ROPE, S], F32R, tag="k_ropeT")
            out_headsT = A.tile([128, HPC, S], BF16, tag="out_headsT")
            cos_t = A.tile([ROPE, S], BF16, tag="cos_t")
            sin_t = A.tile([ROPE, S], BF16, tag="sin_t")
            wuv4_t = A.tile([128, NKVC, HPC * VD], F32R, tag="wuv4")

            # -------- phase 1: c_q^T, c_kv^T, k_rope^T (one hs^T pass) ------
            with tc.tile_pool(name="P1", bufs=1) as P1, \
                 tc.tile_pool(name="P1s", bufs=4) as P1s, \
                 tc.tile_pool(name="P1r", bufs=1) as P1r, \
                 tc.tile_pool(name="PS1", bufs=3, space="PSUM") as PS1, \
                 tc.tile_pool(name="PS1k", bufs=2, space="PSUM") as PS1k:
                qdw_t = P1.tile([128, NDC, QL], F32R, tag="qdw")
                kvdw_t = P1.tile([128, NDC, KVL], F32R, tag="kvdw")
                krw_t = P1.tile([128, NDC, ROPE], F32R, tag="krw")
                nc.sync.dma_start(qdw_t[:, 0, :], qdw_r[:, 0, :])
                nc.sync.dma_start(kvdw_t[:, 0, :], kvdw_r[:, 0, :])
                for sb in range(NSB):
                    ss = bass.ds(sb * 256, 256)
                    ha = P1s.tile([128, 8, 256], F32R, tag="hsT")
                    hb = P1s.tile([128, 8, 256], F32R, tag="hsT")
                    nc.sync.dma_start(ha[:, :, :], hsT_r[:, 0:8, ss])
                    nc.sync.dma_start(hb[:, :, :], hsT_r[:, 8:16, ss])
                    if sb == 0:
                        nc.sync.dma_start(krw_t[:, :, :], krw_r[:, :, :])
                        nc.sync.dma_start(cos_t[:, :], cosd[:, :])
                        nc.sync.dma_start(sin_t[:, :], sind[:, :])
                        for dc in range(1, NDC):
                            nc.sync.dma_start(qdw_t[:, dc, :], qdw_r[:, dc, :])
                            nc.sync.dma_start(kvdw_t[:, dc, :], kvdw_r[:, dc, :])
                        nc.sync.dma_start(wuv4_t[:, :, :], wuv4_r[:, :, :])
                    cq_ps = PS1.tile([128, NQLC, 256], F32, tag="proj")
                    for qlc in range(NQLC):
                        for dc in range(NDC):
                            nc.tensor.matmul(
                                cq_ps[:, qlc, :],
                                qdw_t[:, dc, bass.ts(qlc, 128)],
                                (ha if dc < 8 else hb)[:, dc % 8, :],
                                start=(dc == 0), stop=(dc == NDC - 1))
                    nc.vector.tensor_copy(c_qT[:, :, ss], cq_ps[:, :, :])
                    ckv_ps = PS1.tile([128, NQLC, 256], F32, tag="proj")
                    for qlc in range(NQLC):
                        for dc in range(NDC):
                            nc.tensor.matmul(
                                ckv_ps[:, qlc, :],
                                kvdw_t[:, dc, bass.ts(qlc, 128)],
                                (ha if dc < 8 else hb)[:, dc % 8, :],
                                start=(dc == 0), stop=(dc == NDC - 1))
                    nc.vector.tensor_copy(c_kvT[:, :, ss], ckv_ps[:, :, :])
                    kr_ps = PS1k.tile([ROPE, 256], F32, tag="krp")
                    for dc in range(NDC):
                        nc.tensor.matmul(
                            kr_ps[:, :], krw_t[:, dc, :],
                            (ha if dc < 8 else hb)[:, dc % 8, :],
                            start=(dc == 0), stop=(dc == NDC - 1))
                    kr_raw = P1r.tile([ROPE, 256], F32, tag="kr_raw")
                    nc.vector.tensor_copy(kr_raw[:, :], kr_ps[:, :])
                    _emit_rope(nc, P1r, k_ropeT[:, ss], kr_raw[:, :],
                               cos_t[:, ss], sin_t[:, ss])

            # -------- phase 2: per-head attention --------
            with tc.tile_pool(name="P2", bufs=1) as P2, \
                 tc.tile_pool(name="P2n", bufs=2) as P2n, \
                 tc.tile_pool(name="P2q", bufs=2) as P2q, \
                 tc.tile_pool(name="P2q2", bufs=2) as P2q2, \
                 tc.tile_pool(name="P2v", bufs=1) as P2v, \
                 tc.tile_pool(name="P2e", bufs=4) as P2e, \
                 tc.tile_pool(name="P2r", bufs=1) as P2r, \
                 tc.tile_pool(name="PSmm", bufs=4, space="PSUM") as PSmm, \
                 tc.tile_pool(name="PSqr", bufs=1, space="PSUM") as PSqr, \
                 tc.tile_pool(name="PSov", bufs=2, space="PSUM") as PSov, \
                 tc.tile_pool(name="PSrs", bufs=1, space="PSUM") as PSrs:
                masks_t = P2.tile([128, 4, 512], BF16, tag="masks")
                wukT_t = P2.tile([128, HPC * NQLC, NOPE], F32R, tag="wukT")
                qnw_t = P2.tile([128, NQLC, HPC * NOPE], F32R, tag="qnw")
                qrw_t = P2.tile([128, NQLC, HPC * ROPE], F32R, tag="qrw")
                ones_t = P2.tile([128, 1], BF16, tag="ones")
                nc.sync.dma_start(masks_t[:, :, :], maskd[:, :, :])
                nc.sync.dma_start(wukT_t[:, :, :], wukT_r[:, :, :])
                nc.sync.dma_start(qnw_t[:, :, :], qnw_r[:, :, :])
                nc.sync.dma_start(qrw_t[:, :, :], qrw_r[:, :, :])
                nc.vector.memset(ones_t[:, :], 1.0)

                # absorbed values for all 4 heads in one N=512 pass:
                # vabs4[:, kc, hl*VD+vd] = sum_kv c_kv[k, kv] w_uv[hl*VD+vd, kv]
                vabs4 = P2v.tile([128, NKC, HPC * VD], BF16, tag="vabs")
                for kc in range(NKC):
                    ps4 = PSmm.tile([128, HPC * VD], F32, tag="mm")
                    for kvc in range(NKVC):
                        nc.tensor.matmul(
                            ps4[:, :],
                            c_kvT[:, kvc, bass.ts(kc, 128)],
                            wuv4_t[:, kvc, :],
                            start=(kvc == 0), stop=(kvc == NKVC - 1))
                    if kc % 2 == 0:
                        nc.vector.tensor_copy(vabs4[:, kc, :], ps4[:, :])
                    else:
                        nc.scalar.copy(vabs4[:, kc, :], ps4[:, :])

                self_qr = [None]   # current head's full roped q_rope tile
                self_ka = [None]   # current head's absorbed keys

                def prologue(hl, qb):
                    """q_nope for one (head, 512-wide query block); at qb==0
                    also the head's roped q_rope and absorbed keys
                    k_abs = w_uk_h @ c_kv^T (contracting scores over NOPE=128
                    instead of KVL=512). Returns (qn, k_abs, qr) aps."""
                    qs = bass.ds(qb * 512, 512)
                    qn_qb = P2n.tile([128, 512], F32R, tag="qn")
                    ps = PSmm.tile([128, 512], F32, tag="mm")
                    for qlc in range(NQLC):
                        nc.tensor.matmul(
                            ps[:, :],
                            qnw_t[:, qlc, bass.ds(hl * NOPE, NOPE)],
                            c_qT[:, qlc, qs],
                            start=(qlc == 0), stop=(qlc == NQLC - 1))
                    nc.scalar.copy(qn_qb[:, :], ps[:, :])
                    if qb == 0:
                        # roped q_rope for the WHOLE head, hidden behind the
                        # previous head's attention tail; rope reads PSUM
                        # directly (no raw staging tile)
                        qr_h = P2q2.tile([ROPE, S], F32R, tag="qr_h")
                        for b4 in range(NQB):
                            s4 = bass.ds(b4 * 512, 512)
                            ps2 = PSqr.tile([ROPE, 512], F32, tag="qrps")
                            for qlc in range(NQLC):
                                nc.tensor.matmul(
                                    ps2[:, :],
                                    qrw_t[:, qlc, bass.ds(hl * ROPE, ROPE)],
                                    c_qT[:, qlc, s4],
                                    start=(qlc == 0), stop=(qlc == NQLC - 1))
                            sw = P2q.tile([ROPE, 512], F32, tag="rope_swap")
                            nc.scalar.copy(sw[0:32, :], ps2[32:64, :])
                            nc.scalar.copy(sw[32:64, :], ps2[0:32, :])
                            nc.vector.tensor_mul(qr_h[:, s4], ps2[:, :],
                                                 cos_t[:, s4])
                            nc.vector.tensor_mul(sw[:, :], sw[:, :],
                                                 sin_t[:, s4])
                            nc.vector.tensor_add(
                                qr_h[:, s4], qr_h[:, s4].bitcast(F32),
                                sw[:, :])
                        self_qr[0] = qr_h
                    if qb == 0:
                        kabs = P2q2.tile([128, S], F32R, tag="kabs")
                        for b4 in range(NQB):
                            s4 = bass.ds(b4 * 512, 512)
                            ps3 = PSmm.tile([128, 512], F32, tag="mm")
                            for latc in range(NQLC):
                                nc.tensor.matmul(
                                    ps3[:, :],
                                    wukT_t[:, hl * NQLC + latc, :],
                                    c_kvT[:, latc, s4],
                                    start=(latc == 0), stop=(latc == NQLC - 1))
                            if b4 % 2 == 0:
                                nc.vector.tensor_copy(kabs[:, s4], ps3[:, :])
                            else:
                                nc.scalar.copy(kabs[:, s4], ps3[:, :])
                        self_ka[0] = kabs
                    return (qn_qb, self_ka[0],
                            self_qr[0][:, bass.ds(qb * 512, 512)])

                pairs = [(hl, qb) for hl in range(HPC) for qb in range(NQB)]
                pro = prologue(*pairs[0])
                pending_epi = None    # deferred out_v + normalize of prev pair

                for idx, (hl, qb) in enumerate(pairs):
                    qs = bass.ds(qb * 512, 512)
                    nkc = 4 * qb + 4
                    qn_qb, kabs, qr_qb = pro

                    ov_ps = PSov.tile([128, 512], F32, tag="ov")
                    rs_ps = PSrs.tile([1, 512], F32, tag="rs")
                    pends = []   # deferred exp tiles for PE pipelining

                    def flush(pend, rs_ps=rs_ps, ov_ps=ov_ps, nkc=nkc,
                              hl=hl):
                        e, kc, o = pend
                        nc.tensor.matmul(
                            rs_ps[:, o:512], ones_t[:, :], e[:, o:512],
                            start=(kc == 0), stop=(kc == nkc - 1))
                        nc.tensor.matmul(
                            ov_ps[:, o:512],
                            vabs4[:, kc, bass.ds(hl * VD, VD)],
                            e[:, o:512],
                            start=(kc == 0), stop=(kc == nkc - 1))

                    for kc in range(nkc):
                        # diagonal chunks: skip fully-masked query columns
                        # (width clamped to >=256 to stay in fp32r fast mode)
                        m = kc - 4 * qb
                        o = 0 if m < 0 else min(m * 128, 256)
                        ps_s = PSmm.tile([128, 512], F32, tag="mm")
                        nc.tensor.matmul(
                            ps_s[:, o:512],
                            kabs[:, bass.ts(kc, 128)],
                            qn_qb[:, o:512],
                            start=True, stop=False)
                        nc.tensor.matmul(
                            ps_s[:, o:512],
                            k_ropeT[:, bass.ts(kc, 128)],
                            qr_qb[:, o:512],
                            start=False, stop=True)
                        e = P2e.tile([128, 512], BF16, tag="exp")
                        nc.scalar.activation(e[:, o:512], ps_s[:, o:512],
                                             AF.Exp, scale=SCALE)
                        if m >= 0:
                            # multiplicative causal mask on exp output; sits
                            # off the PSUM-slot critical path (QK->exp)
                            nc.vector.tensor_mul(
                                e[:, o:512], e[:, o:512],
                                masks_t[:, m, o:512])
                        if kc == (3 if nkc == 4 else 5) and pending_epi is not None:
                            # previous pair's out_v runs two score-blocks into
                            # this pair, hiding its ctx copy latency
                            pending_epi()
                            pending_epi = None
                        pends.append((e, kc, o))
                        if len(pends) > 2:
                            flush(pends.pop(0))
                        if kc == max(1, nkc - 3) and idx + 1 < len(pairs):
                            # next pair's q projections: independent PE work
                            # that hides the exp/copy tail of this pair
                            pro = prologue(*pairs[idx + 1])
                    for p in pends:
                        flush(p)
                    pends = []

                    recip = P2r.tile([1, 512], F32, tag="recip")
                    nc.vector.reciprocal(recip[:, :], rs_ps[:, :])
                    rbc = P2r.tile([128, 512], F32, tag="rbc")
                    nc.gpsimd.partition_broadcast(rbc[:, :], recip[:, :])

                    def make_epi(hl=hl, qs=qs, ov_ps=ov_ps, rbc=rbc):
                        def epi():
                            nc.vector.tensor_mul(out_headsT[:, hl, qs],
                                                 ov_ps[:, :], rbc[:, :])
                        return epi

                    pending_epi = make_epi()
                if pending_epi is not None:
                    pending_epi()
                    pending_epi = None

            # -------- phase 3: output projection --------
            with tc.tile_pool(name="P3", bufs=1) as P3, \
                 tc.tile_pool(name="P3s", bufs=8) as P3s, \
                 tc.tile_pool(name="PS3", bufs=6, space="PSUM") as PS3:
                owg_t = P3.tile([128, HPC, D], BF16, tag="owg")
                for hl in range(HPC):
                    nc.sync.dma_start(owg_t[:, hl, :], owg_r[:, hl, :])
                for dc in range(NDC):
                    for qb in range(NQB):
                        qs = bass.ds(qb * 512, 512)
                        ps = PS3.tile([128, 512], F32, tag="op")
                        for hl in range(HPC):
                            nc.tensor.matmul(
                                ps[:, :],
                                owg_t[:, hl, bass.ts(dc, 128)],
                                out_headsT[:, hl, qs],
                                start=(hl == 0), stop=(hl == HPC - 1))
                        st = P3s.tile([128, 512], F32, tag="st")
                        nc.scalar.copy(st[:, :], ps[:, :])
                        nc.sync.dma_start(outT[bass.ts(dc, 128), qs], st[:, :])

    nc.compile()
    return nc


_NC_CACHE = None


def _get_nc():
    global _NC_CACHE
    if _NC_CACHE is None:
        _NC_CACHE = build_nc()
    return _NC_CACHE


def _host_prep(inputs):
    f32 = np.float32
    hs = np.asarray(inputs["hidden_states"], f32)
    qdw = np.ascontiguousarray(np.asarray(inputs["q_down_w"], f32))
    qnw_full = np.asarray(inputs["q_up_nope_w"], f32)
    qrw_full = np.asarray(inputs["q_up_rope_w"], f32)
    kvdw = np.ascontiguousarray(np.asarray(inputs["kv_down_w"], f32))
    krw = np.ascontiguousarray(np.asarray(inputs["k_rope_w"], f32))
    wuk_full = np.asarray(inputs["w_uk"], f32)
    wuv_full = np.asarray(inputs["w_uv"], f32)
    ow = np.asarray(inputs["out_w"], f32)
    cosT, sinT = _rope_tables()
    maskv = _masks()
    hsTs = [np.ascontiguousarray(hs[b].T) for b in range(B)]
    in_maps = []
    for c in range(8):
        b, g = divmod(c, G)
        qnw = np.ascontiguousarray(qnw_full[:, g * HPC * NOPE:(g + 1) * HPC * NOPE])
        qrw = np.ascontiguousarray(qrw_full[:, g * HPC * ROPE:(g + 1) * HPC * ROPE])
        wukg = wuk_full[g * HPC * NOPE:(g + 1) * HPC * NOPE, :]
        wukT = np.ascontiguousarray(np.concatenate(
            [wukg[hl * NOPE:(hl + 1) * NOPE, :].T for hl in range(HPC)], 0))
        wuvg = wuv_full[g * HPC * VD:(g + 1) * HPC * VD, :]
        wuv4 = np.ascontiguousarray(wuvg.T)
        owgv = np.ascontiguousarray(ow[g * HPC * VD:(g + 1) * HPC * VD, :])
        in_maps.append({
            "hsT": hsTs[b],
            "qdw": qdw, "kvdw": kvdw, "krw": krw,
            "qnw": qnw, "qrw": qrw, "wukT": wukT,
            "wuv4": wuv4,
            "owg": owgv.astype(ml_dtypes.bfloat16),
            "cosd": cosT.astype(ml_dtypes.bfloat16),
            "sind": sinT.astype(ml_dtypes.bfloat16),
            "maskd": maskv.astype(ml_dtypes.bfloat16),
        })
    return in_maps


def kernel(**inputs):
    nc = _get_nc()
    in_maps = _host_prep(inputs)
    res = run_bass_kernel_spmd(nc, in_maps, core_ids=list(range(8)))
    out = np.zeros((B, S, D), np.float32)
    for c in range(8):
        out[c // G] += res.results[c]["outT"].T
    out += np.asarray(inputs["out_b"], np.float32)[None, None, :]
    return out



# revision 29
# speedup vs baseline: 1.0881x; 1.0881x over previous
"""DeepSeek-V3 MLA attention kernel for 8 Trainium2 NeuronCores.

Problem: nn_DeepSeekV3_1Attention (B=2, S=2048, D=2048, H=16, NOPE=128,
ROPE=64, VD=128, QL=KVL=512), fp32 reference, causal.

Sharding: data-parallel over batch (2 groups of 4 cores) x tensor-parallel
over heads (4 heads per core). Each core computes its batch's shared
projections (c_q, c_kv, k_rope) redundantly, runs MLA attention for its 4
heads, and produces a partial out-projection (its heads' rows of out_w).
Host sums the 4 partials per batch.

Numerics/performance design (vs the fp32r/bf16 baseline):
 - Phase-1 projections (hs @ {q_down, kv_down, k_rope}) run as 3-term
   fp8e4m3 DoubleRow matmuls: hs and the weights are residual-split on the
   host (x = x1 + x2, both fp8); products x1w1 + x2w1 + x1w2 give
   fp16-class accuracy at 0.75x the fp16 PE cost per pair of 128-deep
   chunks (DR contracts 256 rows at 0.5 cycles/output-col).
 - Everything else runs fp16 (1 cycle/row, better mantissa than bf16).
 - Softmax row-sums are computed TRANSPOSED: matmul(lhsT=exp_tile[:,
   128q], rhs=ones[128,1]) -> out [128q, 1].  Output free size is 1, so
   these matmuls are ~free on the PE; a small PE transpose + reciprocal +
   partition-broadcast restores the row layout for normalization.
 - Unnormalized softmax (exp without max subtraction; scores O(4.6) so
   exp <= ~100 fits fp16), normalization deferred past PV.
"""

import numpy as np
import ml_dtypes

from concourse import bacc
import concourse.bass as bass
import concourse.mybir as mybir
import concourse.tile as tile
from concourse.bass_utils import run_bass_kernel_spmd
from concourse.masks import make_identity

F32 = mybir.dt.float32
F16 = mybir.dt.float16
F8 = mybir.dt.float8e4
AF = mybir.ActivationFunctionType
DR = mybir.MatmulPerfMode.DoubleRow

B, S, D = 2, 2048, 2048
H = 16
NOPE, ROPE, VD = 128, 64, 128
QL, KVL = 512, 512
HPC = 4    # heads per core
G = 4      # cores per batch group
SCALE = float(1.0 / np.sqrt(np.float32(NOPE + ROPE)))

ROPE_WAVELENGTH = 10000.0
ROPE_SCALE = 40.0
BETA_FAST, BETA_SLOW = 32.0, 1.0
OLD_CTX = 4096.0
MSCALE = 1.0
PI = 3.14159265358979

NDC = D // 128          # 16 d-chunks
NDP = NDC // 2          # 8 d-chunk pairs
NQLC = QL // 128        # 4 ql chunks
NKVC = KVL // 128       # 4 kv chunks
NKC = S // 128          # 16 key chunks
NQB = S // 512          # 4 query blocks


def _rope_tables():
    j = np.arange(0, ROPE, 2, dtype=np.float32) / ROPE
    freqs = (1.0 / (ROPE_WAVELENGTH ** j)).astype(np.float32)
    wavelengths = 2.0 * PI / freqs
    ramp = np.clip((wavelengths / OLD_CTX - BETA_SLOW) / (BETA_FAST - BETA_SLOW),
                   0.0, 1.0)
    scale = (1.0 - ramp) + ramp * ROPE_SCALE
    inv_freq = freqs / scale
    t = np.arange(S, dtype=np.float32)
    fr = t[:, None] * inv_freq[None, :]
    cos = (np.cos(fr) * MSCALE).astype(np.float32).T        # [32, S]
    sin = (np.sin(fr) * MSCALE).astype(np.float32).T
    cosT = np.ascontiguousarray(np.concatenate([cos, cos], 0))    # [64, S]
    sinT = np.ascontiguousarray(np.concatenate([-sin, sin], 0))   # [64, S]
    return cosT, sinT


def _masks():
    # multiplicative 0/1 masks applied to exp(scores) on the diagonal chunks
    k = np.arange(128)[:, None]
    q = np.arange(512)[None, :]
    ms = []
    for m in range(4):
        allow = (k + m * 128) <= q
        ms.append(np.where(allow, np.float32(1.0), np.float32(0.0)))
    return np.ascontiguousarray(np.stack(ms, axis=1))    # [128, 4, 512]


def _f8_split(x):
    """x (f32) -> (x1, x2) fp8e4m3 with x ~= x1 + x2."""
    x = np.asarray(x, np.float32)
    x1 = x.astype(ml_dtypes.float8_e4m3)
    x2 = (x - x1.astype(np.float32)).astype(ml_dtypes.float8_e4m3)
    return x1, x2


P1_WSCALE = 64.0   # pre-scale for phase-1 fp8 weights: their natural scale
                   # (~1/sqrt(2048)) sits in e4m3's subnormal range


def _prep_p1_weights(w, m):
    """w [D, m] f32 -> (w1dup [16*2*128, m], w2pair [8*2*128, m]) fp8.

    w1dup[(c, t, p), :]  = w1[128c + p, :]        (t = 0, 1 duplicate)
    w2pair[(j, t, p), :] = w2[128(2j + t) + p, :]
    """
    w1, w2 = _f8_split(np.asarray(w, np.float32) * P1_WSCALE)
    w1c = w1.reshape(NDC, 128, m)
    w1d = np.ascontiguousarray(
        np.broadcast_to(w1c[:, None], (NDC, 2, 128, m))).reshape(-1, m)
    w2p = np.ascontiguousarray(w2.reshape(NDP, 2, 128, m)).reshape(-1, m)
    return w1d, w2p


DEBUG_DUMPS = False
NUM_PHASES = 3


def build_nc():
    nc = bacc.Bacc("TRN2", target_bir_lowering=False, debug=False,
                   enable_asserts=False, num_devices=8)

    hs8 = nc.dram_tensor("hs8", [2 * D, S], F8, kind="ExternalInput").ap()
    qdw1 = nc.dram_tensor("qdw1", [2 * D, QL], F8, kind="ExternalInput").ap()
    qdw2 = nc.dram_tensor("qdw2", [D, QL], F8, kind="ExternalInput").ap()
    kvdw1 = nc.dram_tensor("kvdw1", [2 * D, KVL], F8, kind="ExternalInput").ap()
    kvdw2 = nc.dram_tensor("kvdw2", [D, KVL], F8, kind="ExternalInput").ap()
    krw1 = nc.dram_tensor("krw1", [2 * D, ROPE], F8, kind="ExternalInput").ap()
    krw2 = nc.dram_tensor("krw2", [D, ROPE], F8, kind="ExternalInput").ap()
    qnw = nc.dram_tensor("qnw", [QL, HPC * NOPE], F16, kind="ExternalInput").ap()
    qrw = nc.dram_tensor("qrw", [QL, HPC * ROPE], F16, kind="ExternalInput").ap()
    wukT = nc.dram_tensor("wukT", [HPC * KVL, NOPE], F16, kind="ExternalInput").ap()
    wuv4 = nc.dram_tensor("wuv4", [KVL, HPC * VD], F16, kind="ExternalInput").ap()
    owg = nc.dram_tensor("owg", [HPC * VD, D], F16, kind="ExternalInput").ap()
    cosd = nc.dram_tensor("cosd", [ROPE, S], F16, kind="ExternalInput").ap()
    sind = nc.dram_tensor("sind", [ROPE, S], F16, kind="ExternalInput").ap()
    coskd = nc.dram_tensor("coskd", [ROPE, S], F16, kind="ExternalInput").ap()
    sinkd = nc.dram_tensor("sinkd", [ROPE, S], F16, kind="ExternalInput").ap()
    maskd = nc.dram_tensor("maskd", [128, 4, 512], F16, kind="ExternalInput").ap()
    outT = nc.dram_tensor("outT", [D, S], F16, kind="ExternalOutput").ap()
    if DEBUG_DUMPS:
        cq_dbg = nc.dram_tensor("cq_dbg", [128, NQLC, S], F16,
                                kind="ExternalOutput").ap()
        ckv_dbg = nc.dram_tensor("ckv_dbg", [128, NQLC, S], F16,
                                 kind="ExternalOutput").ap()
        kr_dbg = nc.dram_tensor("kr_dbg", [ROPE, S], F16,
                                kind="ExternalOutput").ap()
        oh_dbg = nc.dram_tensor("oh_dbg", [128, HPC, S], F16,
                                kind="ExternalOutput").ap()

    hs8_r = hs8.rearrange("(t c p) s -> p t c s", t=2, p=128)   # [128,2,16,S]
    qdw1_r = qdw1.rearrange("(c t p) m -> p c t m", t=2, p=128)  # [128,16,2,QL]
    qdw2_r = qdw2.rearrange("(j t p) m -> p j t m", t=2, p=128)  # [128,8,2,QL]
    kvdw1_r = kvdw1.rearrange("(c t p) m -> p c t m", t=2, p=128)
    kvdw2_r = kvdw2.rearrange("(j t p) m -> p j t m", t=2, p=128)
    krw1_r = krw1.rearrange("(c t p) m -> p c t m", t=2, p=128)
    krw2_r = krw2.rearrange("(j t p) m -> p j t m", t=2, p=128)
    qnw_r = qnw.rearrange("(c p) n -> p c n", p=128)      # [128, 4, 512]
    qrw_r = qrw.rearrange("(c p) n -> p c n", p=128)      # [128, 4, 256]
    wukT_r = wukT.rearrange("(c p) n -> p c n", p=128)    # [128, 16, 128]
    wuv4_r = wuv4.rearrange("(c p) v -> p c v", p=128)    # [128, 4, 512]
    owg_r = owg.rearrange("(h p) d -> p h d", p=128)      # [128, 4, D]

    with tile.TileContext(nc) as tc:
        with tc.tile_pool(name="A", bufs=1) as A:
            c_qT = A.tile([128, NQLC, S], F16, tag="c_qT")
            c_kvT = A.tile([128, NQLC, S], F16, tag="c_kvT")
            k_ropeT = A.tile([ROPE, S], F16, tag="k_ropeT")
            out_headsT = A.tile([128, HPC, S], F16, tag="out_headsT")
            cos_t = A.tile([ROPE, S], F16, tag="cos_t")
            sin_t = A.tile([ROPE, S], F16, tag="sin_t")
            cosk_t = A.tile([ROPE, S], F16, tag="cosk_t")
            sink_t = A.tile([ROPE, S], F16, tag="sink_t")
            wuv4_t = A.tile([128, NKVC, HPC * VD], F16, tag="wuv4")
            ident = A.tile([128, 128], F32, tag="ident")
            make_identity(nc, ident[:, :])

            # -------- phase 1: c_q^T, c_kv^T, k_rope^T (one hs^T pass) ------
            # 3-term fp8 DoubleRow: per d-chunk-pair j (chunks c=2j, 2j+1):
            #   A: [x1[c]; x2[c]]     . [w1[c]; w1[c]]
            #   B: [x1[c+1]; x2[c+1]] . [w1[c+1]; w1[c+1]]
            #   C: [x1[c]; x1[c+1]]   . [w2[c]; w2[c+1]]
            with tc.tile_pool(name="P1", bufs=1) as P1, \
                 tc.tile_pool(name="P1s", bufs=6) as P1s, \
                 tc.tile_pool(name="P1r", bufs=1) as P1r, \
                 tc.tile_pool(name="PS1", bufs=5, space="PSUM") as PS1, \
                 tc.tile_pool(name="PS1k", bufs=2, space="PSUM") as PS1k:
                qdw1_t = P1.tile([128, NDC, 2, QL], F8, tag="qdw1")
                qdw2_t = P1.tile([128, NDP, 2, QL], F8, tag="qdw2")
                kvdw1_t = P1.tile([128, NDC, 2, KVL], F8, tag="kvdw1")
                kvdw2_t = P1.tile([128, NDP, 2, KVL], F8, tag="kvdw2")
                krw1_t = P1.tile([128, NDC, 2, ROPE], F8, tag="krw1")
                krw2_t = P1.tile([128, NDP, 2, ROPE], F8, tag="krw2")
                nc.sync.dma_start(qdw1_t[:, 0:8, :, :], qdw1_r[:, 0:8, :, :])
                nc.sync.dma_start(qdw2_t[:, :, :, :], qdw2_r[:, :, :, :])

                def p1_mms(ps, w1_t, w2_t, mslice, ha, hb, half):
                    hs_ = bass.ds(half * 256, 256)
                    for j in range(NDP):
                        c = 2 * j
                        xa = ha if c < 8 else hb
                        xb = ha if c + 1 < 8 else hb
                        st = (j == 0)
                        nc.tensor.matmul(
                            ps[:, :], w1_t[:, c, :, mslice],
                            xa[:, :, c % 8, hs_],
                            start=st, stop=False, perf_mode=DR)
                        nc.tensor.matmul(
                            ps[:, :], w1_t[:, c + 1, :, mslice],
                            xb[:, :, (c + 1) % 8, hs_],
                            start=False, stop=False, perf_mode=DR)
                        if c < 7:   # both chunks of the pair in ha
                            x1p = ha[:, 0, c:c + 2, hs_]
                        else:       # both in hb
                            x1p = hb[:, 0, (c % 8):(c % 8) + 2, hs_]
                        nc.tensor.matmul(
                            ps[:, :], w2_t[:, j, :, mslice], x1p,
                            start=False, stop=(j == NDP - 1), perf_mode=DR)

                for sb in range(NQB):
                    ss = bass.ds(sb * 512, 512)
                    ha = P1s.tile([128, 2, 8, 512], F8, tag="hsT")
                    hb = P1s.tile([128, 2, 8, 512], F8, tag="hsT")
                    for t in range(2):
                        nc.sync.dma_start(ha[:, t, :, :], hs8_r[:, t, 0:8, ss])
                        nc.sync.dma_start(hb[:, t, :, :], hs8_r[:, t, 8:16, ss])
                    if sb == 0:
                        nc.sync.dma_start(qdw1_t[:, 8:16, :, :],
                                          qdw1_r[:, 8:16, :, :])
                        nc.sync.dma_start(kvdw1_t[:, :, :, :],
                                          kvdw1_r[:, :, :, :])
                        nc.sync.dma_start(kvdw2_t[:, :, :, :], kvdw2_r[:, :, :, :])
                        nc.sync.dma_start(krw1_t[:, :, :, :], krw1_r[:, :, :, :])
                        nc.sync.dma_start(krw2_t[:, :, :, :], krw2_r[:, :, :, :])
                        nc.sync.dma_start(cosk_t[:, :], coskd[:, :])
                        nc.sync.dma_start(sink_t[:, :], sinkd[:, :])
                        nc.sync.dma_start(cos_t[:, :], cosd[:, :])
                        nc.sync.dma_start(sin_t[:, :], sind[:, :])
                        nc.sync.dma_start(wuv4_t[:, :, :], wuv4_r[:, :, :])
                    for qlc in range(NQLC):
                        mslice = bass.ts(qlc, 128)
                        cq_ps = PS1.tile([128, 512], F32, tag="proj")
                        for half in range(2):
                            p_s = cq_ps[:, bass.ds(half * 256, 256)]
                            p1_mms(p_s, qdw1_t, qdw2_t, mslice, ha, hb, half)
                        nc.vector.tensor_scalar_mul(c_qT[:, qlc, ss],
                                                    cq_ps[:, :],
                                                    1.0 / P1_WSCALE)
                        ckv_ps = PS1.tile([128, 512], F32, tag="proj")
                        for half in range(2):
                            p_s = ckv_ps[:, bass.ds(half * 256, 256)]
                            p1_mms(p_s, kvdw1_t, kvdw2_t, mslice, ha, hb,
                                   half)
                        nc.scalar.mul(c_kvT[:, qlc, ss], ckv_ps[:, :],
                                      1.0 / P1_WSCALE)
                    kr_ps = PS1k.tile([ROPE, 512], F32, tag="krp")
                    for half in range(2):
                        p_s = kr_ps[:, bass.ds(half * 256, 256)]
                        p1_mms(p_s, krw1_t, krw2_t, slice(0, ROPE), ha, hb,
                               half)
                    # rope rotation, fp16 out
                    sw = P1r.tile([ROPE, 512], F32, tag="kr_sw")
                    kc_ = P1r.tile([ROPE, 512], F32, tag="kr_c")
                    nc.scalar.copy(sw[0:32, :], kr_ps[32:64, :])
                    nc.scalar.copy(sw[32:64, :], kr_ps[0:32, :])
                    nc.vector.tensor_mul(kc_[:, :], kr_ps[:, :], cosk_t[:, ss])
                    nc.vector.tensor_mul(sw[:, :], sw[:, :], sink_t[:, ss])
                    nc.vector.tensor_add(k_ropeT[:, ss], kc_[:, :], sw[:, :])

            if DEBUG_DUMPS:
                nc.sync.dma_start(cq_dbg[:, :, :], c_qT[:, :, :])
                nc.sync.dma_start(ckv_dbg[:, :, :], c_kvT[:, :, :])
                nc.sync.dma_start(kr_dbg[:, :], k_ropeT[:, :])

            # -------- phase 2: per-head attention --------
            with tc.tile_pool(name="P2", bufs=1) as P2, \
                 tc.tile_pool(name="P2n", bufs=2) as P2n, \
                 tc.tile_pool(name="P2q", bufs=2) as P2q, \
                 tc.tile_pool(name="P2q2", bufs=2) as P2q2, \
                 tc.tile_pool(name="P2v", bufs=1) as P2v, \
                 tc.tile_pool(name="P2e", bufs=6) as P2e, \
                 tc.tile_pool(name="P2r", bufs=2) as P2r, \
                 tc.tile_pool(name="PSmm", bufs=4, space="PSUM") as PSmm, \
                 tc.tile_pool(name="PSov", bufs=2, space="PSUM") as PSov, \
                 tc.tile_pool(name="PSrs", bufs=2, space="PSUM") as PSrs:
                masks_t = P2.tile([128, 4, 512], F16, tag="masks")
                wukT_t = P2.tile([128, HPC * NQLC, NOPE], F16, tag="wukT")
                qnw_t = P2.tile([128, NQLC, HPC * NOPE], F16, tag="qnw")
                qrw_t = P2.tile([128, NQLC, HPC * ROPE], F16, tag="qrw")
                ones_t = P2.tile([128, 1], F16, tag="ones")
                nc.sync.dma_start(masks_t[:, :, :], maskd[:, :, :])
                nc.sync.dma_start(wukT_t[:, :, :], wukT_r[:, :, :])
                nc.sync.dma_start(qnw_t[:, :, :], qnw_r[:, :, :])
                nc.sync.dma_start(qrw_t[:, :, :], qrw_r[:, :, :])
                nc.vector.memset(ones_t[:, :], 1.0)

                # absorbed values for all 4 heads in one N=512 pass
                vabs4 = P2v.tile([128, NKC, HPC * VD], F16, tag="vabs")
                for kc in range(NKC):
                    ps4 = PSmm.tile([128, 512], F32, tag="mm")
                    for kvc in range(NKVC):
                        nc.tensor.matmul(
                            ps4[:, :],
                            c_kvT[:, kvc, bass.ts(kc, 128)],
                            wuv4_t[:, kvc, :],
                            start=(kvc == 0), stop=(kvc == NKVC - 1))
                    if kc % 2 == 0:
                        nc.vector.tensor_copy(vabs4[:, kc, :], ps4[:, :])
                    else:
                        nc.scalar.copy(vabs4[:, kc, :], ps4[:, :])

                self_qr = [None]   # current head's full roped q_rope tile
                self_ka = [None]   # current head's absorbed keys

                def prologue(hl, qb):
                    qs = bass.ds(qb * 512, 512)
                    qn_qb = P2n.tile([128, 512], F16, tag="qn")
                    ps = PSmm.tile([128, 512], F32, tag="mm")
                    for qlc in range(NQLC):
                        nc.tensor.matmul(
                            ps[:, :],
                            qnw_t[:, qlc, bass.ds(hl * NOPE, NOPE)],
                            c_qT[:, qlc, qs],
                            start=(qlc == 0), stop=(qlc == NQLC - 1))
                    nc.vector.tensor_copy(qn_qb[:, :], ps[:, :])
                    if qb == 0:
                        # roped q_rope for the WHOLE head
                        qr_h = P2q2.tile([ROPE, S], F16, tag="qr_h")
                        for b4 in range(NQB):
                            s4 = bass.ds(b4 * 512, 512)
                            ps2 = PSmm.tile([128, 512], F32, tag="mm")
                            for qlc in range(NQLC):
                                nc.tensor.matmul(
                                    ps2[0:ROPE, :],
                                    qrw_t[:, qlc, bass.ds(hl * ROPE, ROPE)],
                                    c_qT[:, qlc, s4],
                                    start=(qlc == 0), stop=(qlc == NQLC - 1))
                            sw = P2q.tile([ROPE, 512], F32, tag="rope_swap")
                            qc_ = P2q.tile([ROPE, 512], F32, tag="rope_cos")
                            nc.scalar.copy(sw[0:32, :], ps2[32:64, :])
                            nc.scalar.copy(sw[32:64, :], ps2[0:32, :])
                            nc.vector.tensor_mul(qc_[:, :], ps2[0:ROPE, :],
                                                 cos_t[:, s4])
                            nc.vector.tensor_mul(sw[:, :], sw[:, :],
                                                 sin_t[:, s4])
                            nc.vector.tensor_add(qr_h[:, s4], qc_[:, :],
                                                 sw[:, :])
                        self_qr[0] = qr_h
                    if qb == 0:
                        kabs = P2q2.tile([128, S], F16, tag="kabs")
                        for b4 in range(NQB):
                            s4 = bass.ds(b4 * 512, 512)
                            ps3 = PSmm.tile([128, 512], F32, tag="mm")
                            for latc in range(NQLC):
                                nc.tensor.matmul(
                                    ps3[:, :],
                                    wukT_t[:, hl * NQLC + latc, :],
                                    c_kvT[:, latc, s4],
                                    start=(latc == 0), stop=(latc == NQLC - 1))
                            if b4 % 2 == 0:
                                nc.vector.tensor_copy(kabs[:, s4], ps3[:, :])
                            else:
                                nc.scalar.copy(kabs[:, s4], ps3[:, :])
                        self_ka[0] = kabs
                    return (qn_qb, self_ka[0],
                            self_qr[0][:, bass.ds(qb * 512, 512)])

                pairs = [(hl, qb) for hl in range(HPC) for qb in range(NQB)]
                pro = prologue(*pairs[0])
                pending_epi = None    # deferred out_v + normalize of prev pair

                for idx, (hl, qb) in enumerate(pairs):
                    qs = bass.ds(qb * 512, 512)
                    nkc = 4 * qb + 4
                    qn_qb, kabs, qr_qb = pro

                    ov_ps = PSov.tile([128, 512], F32, tag="ov")
                    rs_ps = PSrs.tile([128, 512], F32, tag="rs")
                    pends = []   # deferred exp tiles for PE pipelining

                    def flush(pend, ov_ps=ov_ps, nkc=nkc, hl=hl):
                        e, kc, o = pend
                        nc.tensor.matmul(
                            ov_ps[:, o:512],
                            vabs4[:, kc, bass.ds(hl * VD, VD)],
                            e[:, o:512],
                            start=(kc == 0), stop=(kc == nkc - 1))

                    first_rs = [True]

                    def rs_mms(pend, rs_ps=rs_ps, qb=qb, first_rs=first_rs):
                        e, kc, o = pend
                        m = kc - 4 * qb
                        for j in range(max(0, m), 4):
                            nc.tensor.matmul(
                                rs_ps[:, j:j + 1],
                                e[:, bass.ts(j, 128)], ones_t[:, :],
                                start=first_rs[0], stop=(kc == 4 * qb + j))
                            first_rs[0] = False

                    for kc in range(nkc):
                        # diagonal chunks: skip fully-masked query columns
                        m = kc - 4 * qb
                        o = 0 if m < 0 else m * 128
                        ps_s = PSmm.tile([128, 512], F32, tag="mm")
                        nc.tensor.matmul(
                            ps_s[:, o:512],
                            kabs[:, bass.ts(kc, 128)],
                            qn_qb[:, o:512],
                            start=True, stop=False)
                        nc.tensor.matmul(
                            ps_s[:, o:512],
                            k_ropeT[:, bass.ts(kc, 128)],
                            qr_qb[:, o:512],
                            start=False, stop=True)
                        e = P2e.tile([128, 512], F16, tag="exp")
                        nc.scalar.activation(e[:, o:512], ps_s[:, o:512],
                                             AF.Exp, scale=SCALE)
                        if m >= 0:
                            nc.vector.tensor_mul(
                                e[:, o:512], e[:, o:512],
                                masks_t[:, m, o:512])
                        rs_mms((e, kc, o))
                        if kc == (3 if nkc == 4 else 5) and pending_epi is not None:
                            pending_epi()
                            pending_epi = None
                        pends.append((e, kc, o))
                        if len(pends) > 3:
                            flush(pends.pop(0))
                        if kc == max(1, nkc - 3) and idx + 1 < len(pairs):
                            pro = prologue(*pairs[idx + 1])
                    for p in pends:
                        flush(p)
                    pends = []

                    # transposed row-sums -> row layout -> reciprocal.
                    # PSUM/partition-base rules: only whole-tile-aligned or
                    # 32-aligned partition starts are valid, so the 4 rs
                    # columns ride at columns {0,32,64,96} of a [128,128]
                    # tile, the transpose puts them on partitions {0,32,64,
                    # 96}, and row extraction happens SBUF-side.
                    rsT_s = P2r.tile([128, 128], F32, tag="rsT")
                    for j in range(4):
                        nc.vector.tensor_copy(rsT_s[:, 32 * j:32 * j + 1],
                                              rs_ps[:, j:j + 1])
                    tr_ps = rs_ps[:, 128:256]
                    nc.tensor.transpose(tr_ps, rsT_s[:, :], ident[:, :])
                    trs = P2r.tile([128, 128], F32, tag="trs")
                    nc.vector.tensor_copy(trs[:, :], tr_ps[:, :])
                    rs_row = P2r.tile([1, 512], F32, tag="rs_row")
                    for j in range(4):
                        nc.gpsimd.tensor_copy(rs_row[0:1, bass.ts(j, 128)],
                                              trs[32 * j:32 * j + 1, :])
                    recip = P2r.tile([1, 512], F32, tag="recip")
                    nc.vector.reciprocal(recip[:, :], rs_row[:, :])
                    rbc = P2r.tile([128, 512], F32, tag="rbc")
                    nc.gpsimd.partition_broadcast(rbc[:, :], recip[:, :])

                    def make_epi(hl=hl, qs=qs, ov_ps=ov_ps, rbc=rbc):
                        def epi():
                            nc.vector.tensor_mul(out_headsT[:, hl, qs],
                                                 ov_ps[:, :], rbc[:, :])
                        return epi

                    pending_epi = make_epi()
                if pending_epi is not None:
                    pending_epi()
                    pending_epi = None

            if DEBUG_DUMPS:
                nc.sync.dma_start(oh_dbg[:, :, :], out_headsT[:, :, :])

            # -------- phase 3: output projection --------
            with tc.tile_pool(name="P3", bufs=1) as P3, \
                 tc.tile_pool(name="P3s", bufs=8) as P3s, \
                 tc.tile_pool(name="PS3", bufs=6, space="PSUM") as PS3:
                owg_t = P3.tile([128, HPC, D], F16, tag="owg")
                for hl in range(HPC):
                    nc.sync.dma_start(owg_t[:, hl, :], owg_r[:, hl, :])
                for dc in range(NDC):
                    for qb in range(NQB):
                        qs = bass.ds(qb * 512, 512)
                        ps = PS3.tile([128, 512], F32, tag="op")
                        for hl in range(HPC):
                            nc.tensor.matmul(
                                ps[:, :],
                                owg_t[:, hl, bass.ts(dc, 128)],
                                out_headsT[:, hl, qs],
                                start=(hl == 0), stop=(hl == HPC - 1))
                        st = P3s.tile([128, 512], F16, tag="st")
                        if (dc * NQB + qb) % 2 == 0:
                            nc.scalar.copy(st[:, :], ps[:, :])
                        else:
                            nc.vector.tensor_copy(st[:, :], ps[:, :])
                        nc.sync.dma_start(outT[bass.ts(dc, 128), qs], st[:, :])

    nc.compile()
    return nc


_NC_CACHE = None


def _get_nc():
    global _NC_CACHE
    if _NC_CACHE is None:
        _NC_CACHE = build_nc()
    return _NC_CACHE


def _host_prep(inputs):
    f32 = np.float32
    f16 = np.float16
    hs = np.asarray(inputs["hidden_states"], f32)
    qnw_full = np.asarray(inputs["q_up_nope_w"], f32)
    qrw_full = np.asarray(inputs["q_up_rope_w"], f32)
    wuk_full = np.asarray(inputs["w_uk"], f32)
    wuv_full = np.asarray(inputs["w_uv"], f32)
    ow = np.asarray(inputs["out_w"], f32)
    cosT, sinT = _rope_tables()
    maskv = _masks()

    qdw1, qdw2 = _prep_p1_weights(np.asarray(inputs["q_down_w"], f32), QL)
    kvdw1, kvdw2 = _prep_p1_weights(np.asarray(inputs["kv_down_w"], f32), KVL)
    krw1, krw2 = _prep_p1_weights(np.asarray(inputs["k_rope_w"], f32), ROPE)

    # hs8: [2(term), 16(chunk), 128, S] per batch
    hs8s = []
    for b in range(B):
        hsT = np.ascontiguousarray(hs[b].T)                  # [D, S]
        h1, h2 = _f8_split(hsT)
        hs8s.append(np.ascontiguousarray(
            np.stack([h1.reshape(NDC, 128, S), h2.reshape(NDC, 128, S)], 0)
        ).reshape(2 * D, S))

    in_maps = []
    for c in range(8):
        b, g = divmod(c, G)
        qnwg = np.ascontiguousarray(
            qnw_full[:, g * HPC * NOPE:(g + 1) * HPC * NOPE]).astype(f16)
        qrwg = np.ascontiguousarray(
            qrw_full[:, g * HPC * ROPE:(g + 1) * HPC * ROPE]).astype(f16)
        wukg = wuk_full[g * HPC * NOPE:(g + 1) * HPC * NOPE, :]
        wukTg = np.ascontiguousarray(np.concatenate(
            [wukg[hl * NOPE:(hl + 1) * NOPE, :].T for hl in range(HPC)],
            0)).astype(f16)
        wuvg = wuv_full[g * HPC * VD:(g + 1) * HPC * VD, :]
        wuv4g = np.ascontiguousarray(wuvg.T).astype(f16)
        owgv = np.ascontiguousarray(
            ow[g * HPC * VD:(g + 1) * HPC * VD, :]).astype(f16)
        in_maps.append({
            "hs8": hs8s[b],
            "qdw1": qdw1, "qdw2": qdw2,
            "kvdw1": kvdw1, "kvdw2": kvdw2,
            "krw1": krw1, "krw2": krw2,
            "qnw": qnwg, "qrw": qrwg, "wukT": wukTg, "wuv4": wuv4g,
            "owg": owgv,
            "cosd": cosT.astype(f16), "sind": sinT.astype(f16),
            "coskd": (cosT / P1_WSCALE).astype(f16),
            "sinkd": (sinT / P1_WSCALE).astype(f16),
            "maskd": maskv.astype(f16),
        })
    return in_maps


def kernel(**inputs):
    nc = _get_nc()
    in_maps = _host_prep(inputs)
    res = run_bass_kernel_spmd(nc, in_maps, core_ids=list(range(8)))
    out = np.zeros((B, S, D), np.float32)
    for c in range(8):
        out[c // G] += res.results[c]["outT"].astype(np.float32).T
    out += np.asarray(inputs["out_b"], np.float32)[None, None, :]
    return out


# revision 30
# speedup vs baseline: 1.1224x; 1.0316x over previous
"""DeepSeek-V3 MLA attention kernel for 8 Trainium2 NeuronCores.

Problem: nn_DeepSeekV3_1Attention (B=2, S=2048, D=2048, H=16, NOPE=128,
ROPE=64, VD=128, QL=KVL=512), fp32 reference, causal.

Sharding: data-parallel over batch (2 groups of 4 cores) x tensor-parallel
over heads (4 heads per core). Each core computes its batch's shared
projections (c_q, c_kv, k_rope) redundantly, runs MLA attention for its 4
heads, and produces a partial out-projection (its heads' rows of out_w).
Host sums the 4 partials per batch.

Numerics/performance design (vs the fp32r/bf16 baseline):
 - Phase-1 projections (hs @ {q_down, kv_down, k_rope}) run as 3-term
   fp8e4m3 DoubleRow matmuls: hs and the weights are residual-split on the
   host (x = x1 + x2, both fp8); products x1w1 + x2w1 + x1w2 give
   fp16-class accuracy at 0.75x the fp16 PE cost per pair of 128-deep
   chunks (DR contracts 256 rows at 0.5 cycles/output-col).
 - Everything else runs fp16 (1 cycle/row, better mantissa than bf16).
 - Softmax row-sums are computed TRANSPOSED: matmul(lhsT=exp_tile[:,
   128q], rhs=ones[128,1]) -> out [128q, 1].  Output free size is 1, so
   these matmuls are ~free on the PE; a small PE transpose + reciprocal +
   partition-broadcast restores the row layout for normalization.
 - Unnormalized softmax (exp without max subtraction; scores O(4.6) so
   exp <= ~100 fits fp16), normalization deferred past PV.
"""

import numpy as np
import ml_dtypes

from concourse import bacc
import concourse.bass as bass
import concourse.mybir as mybir
import concourse.tile as tile
from concourse.bass_utils import run_bass_kernel_spmd
from concourse.masks import make_identity

F32 = mybir.dt.float32
F16 = mybir.dt.float16
F8 = mybir.dt.float8e4
AF = mybir.ActivationFunctionType
DR = mybir.MatmulPerfMode.DoubleRow

B, S, D = 2, 2048, 2048
H = 16
NOPE, ROPE, VD = 128, 64, 128
QL, KVL = 512, 512
HPC = 4    # heads per core
G = 4      # cores per batch group
SCALE = float(1.0 / np.sqrt(np.float32(NOPE + ROPE)))

ROPE_WAVELENGTH = 10000.0
ROPE_SCALE = 40.0
BETA_FAST, BETA_SLOW = 32.0, 1.0
OLD_CTX = 4096.0
MSCALE = 1.0
PI = 3.14159265358979

NDC = D // 128          # 16 d-chunks
NDP = NDC // 2          # 8 d-chunk pairs
NQLC = QL // 128        # 4 ql chunks
NKVC = KVL // 128       # 4 kv chunks
NKC = S // 128          # 16 key chunks
NQB = S // 512          # 4 query blocks


def _rope_tables():
    j = np.arange(0, ROPE, 2, dtype=np.float32) / ROPE
    freqs = (1.0 / (ROPE_WAVELENGTH ** j)).astype(np.float32)
    wavelengths = 2.0 * PI / freqs
    ramp = np.clip((wavelengths / OLD_CTX - BETA_SLOW) / (BETA_FAST - BETA_SLOW),
                   0.0, 1.0)
    scale = (1.0 - ramp) + ramp * ROPE_SCALE
    inv_freq = freqs / scale
    t = np.arange(S, dtype=np.float32)
    fr = t[:, None] * inv_freq[None, :]
    cos = (np.cos(fr) * MSCALE).astype(np.float32).T        # [32, S]
    sin = (np.sin(fr) * MSCALE).astype(np.float32).T
    cosT = np.ascontiguousarray(np.concatenate([cos, cos], 0))    # [64, S]
    sinT = np.ascontiguousarray(np.concatenate([-sin, sin], 0))   # [64, S]
    return cosT, sinT


def _masks():
    # multiplicative 0/1 masks applied to exp(scores) on the diagonal chunks
    k = np.arange(128)[:, None]
    q = np.arange(512)[None, :]
    ms = []
    for m in range(4):
        allow = (k + m * 128) <= q
        ms.append(np.where(allow, np.float32(1.0), np.float32(0.0)))
    return np.ascontiguousarray(np.stack(ms, axis=1))    # [128, 4, 512]


def _f8_split(x):
    """x (f32) -> (x1, x2) fp8e4m3 with x ~= x1 + x2."""
    x = np.asarray(x, np.float32)
    x1 = x.astype(ml_dtypes.float8_e4m3)
    x2 = (x - x1.astype(np.float32)).astype(ml_dtypes.float8_e4m3)
    return x1, x2


P1_WSCALE = 64.0   # pre-scale for phase-1 fp8 weights: their natural scale
                   # (~1/sqrt(2048)) sits in e4m3's subnormal range


def _prep_p1_weights(w, m):
    """w [D, m] f32 -> (w1dup [16*2*128, m], w2pair [8*2*128, m]) fp8.

    w1dup[(c, t, p), :]  = w1[128c + p, :]        (t = 0, 1 duplicate)
    w2pair[(j, t, p), :] = w2[128(2j + t) + p, :]
    """
    w1, w2 = _f8_split(np.asarray(w, np.float32) * P1_WSCALE)
    w1c = w1.reshape(NDC, 128, m)
    w1d = np.ascontiguousarray(
        np.broadcast_to(w1c[:, None], (NDC, 2, 128, m))).reshape(-1, m)
    w2p = np.ascontiguousarray(w2.reshape(NDP, 2, 128, m)).reshape(-1, m)
    return w1d, w2p


DEBUG_DUMPS = False
NUM_PHASES = 3


def build_nc():
    nc = bacc.Bacc("TRN2", target_bir_lowering=False, debug=False,
                   enable_asserts=False, num_devices=8)

    hs8 = nc.dram_tensor("hs8", [2 * D, S], F8, kind="ExternalInput").ap()
    qdw1 = nc.dram_tensor("qdw1", [2 * D, QL], F8, kind="ExternalInput").ap()
    qdw2 = nc.dram_tensor("qdw2", [D, QL], F8, kind="ExternalInput").ap()
    kvdw1 = nc.dram_tensor("kvdw1", [2 * D, KVL], F8, kind="ExternalInput").ap()
    kvdw2 = nc.dram_tensor("kvdw2", [D, KVL], F8, kind="ExternalInput").ap()
    krw1 = nc.dram_tensor("krw1", [2 * D, ROPE], F8, kind="ExternalInput").ap()
    krw2 = nc.dram_tensor("krw2", [D, ROPE], F8, kind="ExternalInput").ap()
    qnw = nc.dram_tensor("qnw", [QL, HPC * NOPE], F16, kind="ExternalInput").ap()
    qrw = nc.dram_tensor("qrw", [QL, HPC * ROPE], F16, kind="ExternalInput").ap()
    wukT = nc.dram_tensor("wukT", [HPC * KVL, NOPE], F16, kind="ExternalInput").ap()
    wuv4 = nc.dram_tensor("wuv4", [KVL, HPC * VD], F16, kind="ExternalInput").ap()
    owg = nc.dram_tensor("owg", [HPC * VD, D], F16, kind="ExternalInput").ap()
    cosd = nc.dram_tensor("cosd", [ROPE, S], F16, kind="ExternalInput").ap()
    sind = nc.dram_tensor("sind", [ROPE, S], F16, kind="ExternalInput").ap()
    coskd = nc.dram_tensor("coskd", [ROPE, S], F16, kind="ExternalInput").ap()
    sinkd = nc.dram_tensor("sinkd", [ROPE, S], F16, kind="ExternalInput").ap()
    maskd = nc.dram_tensor("maskd", [128, 4, 512], F16, kind="ExternalInput").ap()
    outT = nc.dram_tensor("outT", [D, S], F16, kind="ExternalOutput").ap()
    if DEBUG_DUMPS:
        cq_dbg = nc.dram_tensor("cq_dbg", [128, NQLC, S], F16,
                                kind="ExternalOutput").ap()
        ckv_dbg = nc.dram_tensor("ckv_dbg", [128, NQLC, S], F16,
                                 kind="ExternalOutput").ap()
        kr_dbg = nc.dram_tensor("kr_dbg", [ROPE, S], F16,
                                kind="ExternalOutput").ap()
        oh_dbg = nc.dram_tensor("oh_dbg", [128, HPC, S], F16,
                                kind="ExternalOutput").ap()

    hs8_r = hs8.rearrange("(t c p) s -> p t c s", t=2, p=128)   # [128,2,16,S]
    qdw1_r = qdw1.rearrange("(c t p) m -> p c t m", t=2, p=128)  # [128,16,2,QL]
    qdw2_r = qdw2.rearrange("(j t p) m -> p j t m", t=2, p=128)  # [128,8,2,QL]
    kvdw1_r = kvdw1.rearrange("(c t p) m -> p c t m", t=2, p=128)
    kvdw2_r = kvdw2.rearrange("(j t p) m -> p j t m", t=2, p=128)
    krw1_r = krw1.rearrange("(c t p) m -> p c t m", t=2, p=128)
    krw2_r = krw2.rearrange("(j t p) m -> p j t m", t=2, p=128)
    qnw_r = qnw.rearrange("(c p) n -> p c n", p=128)      # [128, 4, 512]
    qrw_r = qrw.rearrange("(c p) n -> p c n", p=128)      # [128, 4, 256]
    wukT_r = wukT.rearrange("(c p) n -> p c n", p=128)    # [128, 16, 128]
    wuv4_r = wuv4.rearrange("(c p) v -> p c v", p=128)    # [128, 4, 512]
    owg_r = owg.rearrange("(h p) d -> p h d", p=128)      # [128, 4, D]

    with tile.TileContext(nc) as tc:
        with tc.tile_pool(name="A", bufs=1) as A:
            c_qT = A.tile([128, NQLC, S], F16, tag="c_qT")
            c_kvT = A.tile([128, NQLC, S], F16, tag="c_kvT")
            k_ropeT = A.tile([ROPE, S], F16, tag="k_ropeT")
            out_headsT = A.tile([128, HPC, S], F16, tag="out_headsT")
            cos_t = A.tile([ROPE, S], F16, tag="cos_t")
            sin_t = A.tile([ROPE, S], F16, tag="sin_t")
            cosk_t = A.tile([ROPE, S], F16, tag="cosk_t")
            sink_t = A.tile([ROPE, S], F16, tag="sink_t")
            wuv4_t = A.tile([128, NKVC, HPC * VD], F16, tag="wuv4")
            owg_t = A.tile([128, HPC, D], F16, tag="owg")
            ident = A.tile([128, 128], F32, tag="ident")
            make_identity(nc, ident[:, :])

            # -------- phase 1: c_q^T, c_kv^T, k_rope^T (one hs^T pass) ------
            # 3-term fp8 DoubleRow: per d-chunk-pair j (chunks c=2j, 2j+1):
            #   A: [x1[c]; x2[c]]     . [w1[c]; w1[c]]
            #   B: [x1[c+1]; x2[c+1]] . [w1[c+1]; w1[c+1]]
            #   C: [x1[c]; x1[c+1]]   . [w2[c]; w2[c+1]]
            with tc.tile_pool(name="P1", bufs=1) as P1, \
                 tc.tile_pool(name="P1s", bufs=6) as P1s, \
                 tc.tile_pool(name="P1r", bufs=1) as P1r, \
                 tc.tile_pool(name="PS1", bufs=5, space="PSUM") as PS1, \
                 tc.tile_pool(name="PS1k", bufs=2, space="PSUM") as PS1k:
                qdw1_t = P1.tile([128, NDC, 2, QL], F8, tag="qdw1")
                qdw2_t = P1.tile([128, NDP, 2, QL], F8, tag="qdw2")
                kvdw1_t = P1.tile([128, NDC, 2, KVL], F8, tag="kvdw1")
                kvdw2_t = P1.tile([128, NDP, 2, KVL], F8, tag="kvdw2")
                krw1_t = P1.tile([128, NDC, 2, ROPE], F8, tag="krw1")
                krw2_t = P1.tile([128, NDP, 2, ROPE], F8, tag="krw2")
                nc.sync.dma_start(qdw1_t[:, 0:8, :, :], qdw1_r[:, 0:8, :, :])
                nc.sync.dma_start(qdw2_t[:, :, :, :], qdw2_r[:, :, :, :])

                def p1_mms(ps, w1_t, w2_t, mslice, ha, hb, half):
                    hs_ = bass.ds(half * 256, 256)
                    for j in range(NDP):
                        c = 2 * j
                        xa = ha if c < 8 else hb
                        xb = ha if c + 1 < 8 else hb
                        st = (j == 0)
                        nc.tensor.matmul(
                            ps[:, :], w1_t[:, c, :, mslice],
                            xa[:, :, c % 8, hs_],
                            start=st, stop=False, perf_mode=DR)
                        nc.tensor.matmul(
                            ps[:, :], w1_t[:, c + 1, :, mslice],
                            xb[:, :, (c + 1) % 8, hs_],
                            start=False, stop=False, perf_mode=DR)
                        if c < 7:   # both chunks of the pair in ha
                            x1p = ha[:, 0, c:c + 2, hs_]
                        else:       # both in hb
                            x1p = hb[:, 0, (c % 8):(c % 8) + 2, hs_]
                        nc.tensor.matmul(
                            ps[:, :], w2_t[:, j, :, mslice], x1p,
                            start=False, stop=(j == NDP - 1), perf_mode=DR)

                for sb in range(NQB):
                    ss = bass.ds(sb * 512, 512)
                    ha = P1s.tile([128, 2, 8, 512], F8, tag="hsT")
                    hb = P1s.tile([128, 2, 8, 512], F8, tag="hsT")
                    for t in range(2):
                        nc.sync.dma_start(ha[:, t, :, :], hs8_r[:, t, 0:8, ss])
                        nc.sync.dma_start(hb[:, t, :, :], hs8_r[:, t, 8:16, ss])
                    if sb == 0:
                        nc.sync.dma_start(qdw1_t[:, 8:16, :, :],
                                          qdw1_r[:, 8:16, :, :])
                        nc.sync.dma_start(kvdw1_t[:, :, :, :],
                                          kvdw1_r[:, :, :, :])
                        nc.sync.dma_start(kvdw2_t[:, :, :, :], kvdw2_r[:, :, :, :])
                        nc.sync.dma_start(krw1_t[:, :, :, :], krw1_r[:, :, :, :])
                        nc.sync.dma_start(krw2_t[:, :, :, :], krw2_r[:, :, :, :])
                        nc.sync.dma_start(cosk_t[:, :], coskd[:, :])
                        nc.sync.dma_start(sink_t[:, :], sinkd[:, :])
                        nc.sync.dma_start(cos_t[:, :], cosd[:, :])
                        nc.sync.dma_start(sin_t[:, :], sind[:, :])
                        nc.sync.dma_start(wuv4_t[:, :, :], wuv4_r[:, :, :])
                    for qlc in range(NQLC):
                        mslice = bass.ts(qlc, 128)
                        cq_ps = PS1.tile([128, 512], F32, tag="proj")
                        for half in range(2):
                            p_s = cq_ps[:, bass.ds(half * 256, 256)]
                            p1_mms(p_s, qdw1_t, qdw2_t, mslice, ha, hb, half)
                        nc.vector.tensor_scalar_mul(c_qT[:, qlc, ss],
                                                    cq_ps[:, :],
                                                    1.0 / P1_WSCALE)
                        ckv_ps = PS1.tile([128, 512], F32, tag="proj")
                        for half in range(2):
                            p_s = ckv_ps[:, bass.ds(half * 256, 256)]
                            p1_mms(p_s, kvdw1_t, kvdw2_t, mslice, ha, hb,
                                   half)
                        nc.scalar.mul(c_kvT[:, qlc, ss], ckv_ps[:, :],
                                      1.0 / P1_WSCALE)
                    kr_ps = PS1k.tile([ROPE, 512], F32, tag="krp")
                    for half in range(2):
                        p_s = kr_ps[:, bass.ds(half * 256, 256)]
                        p1_mms(p_s, krw1_t, krw2_t, slice(0, ROPE), ha, hb,
                               half)
                    # rope rotation, fp16 out
                    sw = P1r.tile([ROPE, 512], F32, tag="kr_sw")
                    kc_ = P1r.tile([ROPE, 512], F32, tag="kr_c")
                    nc.scalar.copy(sw[0:32, :], kr_ps[32:64, :])
                    nc.scalar.copy(sw[32:64, :], kr_ps[0:32, :])
                    nc.vector.tensor_mul(kc_[:, :], kr_ps[:, :], cosk_t[:, ss])
                    nc.vector.tensor_mul(sw[:, :], sw[:, :], sink_t[:, ss])
                    nc.vector.tensor_add(k_ropeT[:, ss], kc_[:, :], sw[:, :])

            if DEBUG_DUMPS:
                nc.sync.dma_start(cq_dbg[:, :, :], c_qT[:, :, :])
                nc.sync.dma_start(ckv_dbg[:, :, :], c_kvT[:, :, :])
                nc.sync.dma_start(kr_dbg[:, :], k_ropeT[:, :])

            # -------- phase 2: per-head attention --------
            with tc.tile_pool(name="P2", bufs=1) as P2, \
                 tc.tile_pool(name="P2n", bufs=2) as P2n, \
                 tc.tile_pool(name="P2q", bufs=2) as P2q, \
                 tc.tile_pool(name="P2q2", bufs=2) as P2q2, \
                 tc.tile_pool(name="P2v", bufs=1) as P2v, \
                 tc.tile_pool(name="P2e", bufs=6) as P2e, \
                 tc.tile_pool(name="P2r", bufs=2) as P2r, \
                 tc.tile_pool(name="PSmm", bufs=4, space="PSUM") as PSmm, \
                 tc.tile_pool(name="PSov", bufs=2, space="PSUM") as PSov, \
                 tc.tile_pool(name="PSrs", bufs=2, space="PSUM") as PSrs:
                masks_t = P2.tile([128, 4, 512], F16, tag="masks")
                wukT_t = P2.tile([128, HPC * NQLC, NOPE], F16, tag="wukT")
                qnw_t = P2.tile([128, NQLC, HPC * NOPE], F16, tag="qnw")
                qrw_t = P2.tile([128, NQLC, HPC * ROPE], F16, tag="qrw")
                ones_t = P2.tile([128, 1], F16, tag="ones")
                nc.sync.dma_start(masks_t[:, :, :], maskd[:, :, :])
                for hl in range(HPC):
                    nc.sync.dma_start(owg_t[:, hl, :], owg_r[:, hl, :])
                nc.sync.dma_start(wukT_t[:, :, :], wukT_r[:, :, :])
                nc.sync.dma_start(qnw_t[:, :, :], qnw_r[:, :, :])
                nc.sync.dma_start(qrw_t[:, :, :], qrw_r[:, :, :])
                nc.vector.memset(ones_t[:, :], 1.0)

                # absorbed values for all 4 heads in one N=512 pass
                vabs4 = P2v.tile([128, NKC, HPC * VD], F16, tag="vabs")
                for kc in range(NKC):
                    ps4 = PSmm.tile([128, 512], F32, tag="mm")
                    for kvc in range(NKVC):
                        nc.tensor.matmul(
                            ps4[:, :],
                            c_kvT[:, kvc, bass.ts(kc, 128)],
                            wuv4_t[:, kvc, :],
                            start=(kvc == 0), stop=(kvc == NKVC - 1))
                    if kc % 2 == 0:
                        nc.vector.tensor_copy(vabs4[:, kc, :], ps4[:, :])
                    else:
                        nc.scalar.copy(vabs4[:, kc, :], ps4[:, :])

                self_qr = [None]   # current head's full roped q_rope tile
                self_ka = [None]   # current head's absorbed keys

                def prologue(hl, qb):
                    qs = bass.ds(qb * 512, 512)
                    qn_qb = P2n.tile([128, 512], F16, tag="qn")
                    ps = PSmm.tile([128, 512], F32, tag="mm")
                    for qlc in range(NQLC):
                        nc.tensor.matmul(
                            ps[:, :],
                            qnw_t[:, qlc, bass.ds(hl * NOPE, NOPE)],
                            c_qT[:, qlc, qs],
                            start=(qlc == 0), stop=(qlc == NQLC - 1))
                    nc.vector.tensor_copy(qn_qb[:, :], ps[:, :])
                    if qb == 0:
                        # roped q_rope for the WHOLE head
                        qr_h = P2q2.tile([ROPE, S], F16, tag="qr_h")
                        for b4 in range(NQB):
                            s4 = bass.ds(b4 * 512, 512)
                            ps2 = PSmm.tile([128, 512], F32, tag="mm")
                            for qlc in range(NQLC):
                                nc.tensor.matmul(
                                    ps2[0:ROPE, :],
                                    qrw_t[:, qlc, bass.ds(hl * ROPE, ROPE)],
                                    c_qT[:, qlc, s4],
                                    start=(qlc == 0), stop=(qlc == NQLC - 1))
                            sw = P2q.tile([ROPE, 512], F32, tag="rope_swap")
                            qc_ = P2q.tile([ROPE, 512], F32, tag="rope_cos")
                            nc.scalar.copy(sw[0:32, :], ps2[32:64, :])
                            nc.scalar.copy(sw[32:64, :], ps2[0:32, :])
                            nc.vector.tensor_mul(qc_[:, :], ps2[0:ROPE, :],
                                                 cos_t[:, s4])
                            nc.vector.tensor_mul(sw[:, :], sw[:, :],
                                                 sin_t[:, s4])
                            nc.vector.tensor_add(qr_h[:, s4], qc_[:, :],
                                                 sw[:, :])
                        self_qr[0] = qr_h
                    if qb == 0:
                        kabs = P2q2.tile([128, S], F16, tag="kabs")
                        for b4 in range(NQB):
                            s4 = bass.ds(b4 * 512, 512)
                            ps3 = PSmm.tile([128, 512], F32, tag="mm")
                            for latc in range(NQLC):
                                nc.tensor.matmul(
                                    ps3[:, :],
                                    wukT_t[:, hl * NQLC + latc, :],
                                    c_kvT[:, latc, s4],
                                    start=(latc == 0), stop=(latc == NQLC - 1))
                            if b4 % 2 == 0:
                                nc.vector.tensor_copy(kabs[:, s4], ps3[:, :])
                            else:
                                nc.scalar.copy(kabs[:, s4], ps3[:, :])
                        self_ka[0] = kabs
                    return (qn_qb, self_ka[0],
                            self_qr[0][:, bass.ds(qb * 512, 512)])

                pairs = [(hl, qb) for hl in range(HPC) for qb in range(NQB)]
                pro = prologue(*pairs[0])
                pending_epi = None    # deferred out_v + normalize of prev pair

                for idx, (hl, qb) in enumerate(pairs):
                    qs = bass.ds(qb * 512, 512)
                    nkc = 4 * qb + 4
                    qn_qb, kabs, qr_qb = pro

                    ov_ps = PSov.tile([128, 512], F32, tag="ov")
                    rs_ps = PSrs.tile([128, 512], F32, tag="rs")
                    pends = []   # deferred exp tiles for PE pipelining

                    def flush(pend, ov_ps=ov_ps, nkc=nkc, hl=hl):
                        e, kc, o = pend
                        nc.tensor.matmul(
                            ov_ps[:, o:512],
                            vabs4[:, kc, bass.ds(hl * VD, VD)],
                            e[:, o:512],
                            start=(kc == 0), stop=(kc == nkc - 1))

                    first_rs = [True]

                    def rs_mms(pend, rs_ps=rs_ps, qb=qb, first_rs=first_rs):
                        e, kc, o = pend
                        m = kc - 4 * qb
                        for j in range(max(0, m), 4):
                            nc.tensor.matmul(
                                rs_ps[:, j:j + 1],
                                e[:, bass.ts(j, 128)], ones_t[:, :],
                                start=first_rs[0], stop=(kc == 4 * qb + j))
                            first_rs[0] = False

                    for kc in range(nkc):
                        # diagonal chunks: skip fully-masked query columns
                        m = kc - 4 * qb
                        o = 0 if m < 0 else m * 128
                        ps_s = PSmm.tile([128, 512], F32, tag="mm")
                        nc.tensor.matmul(
                            ps_s[:, o:512],
                            kabs[:, bass.ts(kc, 128)],
                            qn_qb[:, o:512],
                            start=True, stop=False)
                        nc.tensor.matmul(
                            ps_s[:, o:512],
                            k_ropeT[:, bass.ts(kc, 128)],
                            qr_qb[:, o:512],
                            start=False, stop=True)
                        e = P2e.tile([128, 512], F16, tag="exp")
                        nc.scalar.activation(e[:, o:512], ps_s[:, o:512],
                                             AF.Exp, scale=SCALE)
                        if m >= 0:
                            nc.vector.tensor_mul(
                                e[:, o:512], e[:, o:512],
                                masks_t[:, m, o:512])
                        rs_mms((e, kc, o))
                        if kc == (3 if nkc == 4 else 5) and pending_epi is not None:
                            pending_epi()
                            pending_epi = None
                        pends.append((e, kc, o))
                        if len(pends) > 3:
                            flush(pends.pop(0))
                        if kc == max(1, nkc - 3) and idx + 1 < len(pairs):
                            pro = prologue(*pairs[idx + 1])
                    for p in pends:
                        flush(p)
                    pends = []

                    # transposed row-sums -> row layout -> reciprocal.
                    # PSUM/partition-base rules: only whole-tile-aligned or
                    # 32-aligned partition starts are valid, so the 4 rs
                    # columns ride at columns {0,32,64,96} of a [128,128]
                    # tile, the transpose puts them on partitions {0,32,64,
                    # 96}, and row extraction happens SBUF-side.
                    rsT_s = P2r.tile([128, 128], F32, tag="rsT")
                    for j in range(4):
                        nc.vector.tensor_copy(rsT_s[:, 32 * j:32 * j + 1],
                                              rs_ps[:, j:j + 1])
                    tr_ps = rs_ps[:, 128:256]
                    nc.tensor.transpose(tr_ps, rsT_s[:, :], ident[:, :])
                    trs = P2r.tile([128, 128], F32, tag="trs")
                    nc.vector.tensor_copy(trs[:, :], tr_ps[:, :])
                    rs_row = P2r.tile([1, 512], F32, tag="rs_row")
                    for j in range(4):
                        nc.gpsimd.tensor_copy(rs_row[0:1, bass.ts(j, 128)],
                                              trs[32 * j:32 * j + 1, :])
                    recip = P2r.tile([1, 512], F32, tag="recip")
                    nc.vector.reciprocal(recip[:, :], rs_row[:, :])
                    rbc = P2r.tile([128, 512], F32, tag="rbc")
                    nc.gpsimd.partition_broadcast(rbc[:, :], recip[:, :])

                    def make_epi(hl=hl, qs=qs, ov_ps=ov_ps, rbc=rbc):
                        def epi():
                            nc.vector.tensor_mul(out_headsT[:, hl, qs],
                                                 ov_ps[:, :], rbc[:, :])
                        return epi

                    pending_epi = make_epi()
                if pending_epi is not None:
                    pending_epi()
                    pending_epi = None

            if DEBUG_DUMPS:
                nc.sync.dma_start(oh_dbg[:, :, :], out_headsT[:, :, :])

            # -------- phase 3: output projection --------
            with tc.tile_pool(name="P3s", bufs=8) as P3s, \
                 tc.tile_pool(name="PS3", bufs=6, space="PSUM") as PS3:
                for dc in range(NDC):
                    for qb in range(NQB):
                        qs = bass.ds(qb * 512, 512)
                        ps = PS3.tile([128, 512], F32, tag="op")
                        for hl in range(HPC):
                            nc.tensor.matmul(
                                ps[:, :],
                                owg_t[:, hl, bass.ts(dc, 128)],
                                out_headsT[:, hl, qs],
                                start=(hl == 0), stop=(hl == HPC - 1))
                        st = P3s.tile([128, 512], F16, tag="st")
                        if (dc * NQB + qb) % 2 == 0:
                            nc.scalar.copy(st[:, :], ps[:, :])
                        else:
                            nc.vector.tensor_copy(st[:, :], ps[:, :])
                        nc.sync.dma_start(outT[bass.ts(dc, 128), qs], st[:, :])

    nc.compile()
    return nc


_NC_CACHE = None


def _get_nc():
    global _NC_CACHE
    if _NC_CACHE is None:
        _NC_CACHE = build_nc()
    return _NC_CACHE


def _host_prep(inputs):
    f32 = np.float32
    f16 = np.float16
    hs = np.asarray(inputs["hidden_states"], f32)
    qnw_full = np.asarray(inputs["q_up_nope_w"], f32)
    qrw_full = np.asarray(inputs["q_up_rope_w"], f32)
    wuk_full = np.asarray(inputs["w_uk"], f32)
    wuv_full = np.asarray(inputs["w_uv"], f32)
    ow = np.asarray(inputs["out_w"], f32)
    cosT, sinT = _rope_tables()
    maskv = _masks()

    qdw1, qdw2 = _prep_p1_weights(np.asarray(inputs["q_down_w"], f32), QL)
    kvdw1, kvdw2 = _prep_p1_weights(np.asarray(inputs["kv_down_w"], f32), KVL)
    krw1, krw2 = _prep_p1_weights(np.asarray(inputs["k_rope_w"], f32), ROPE)

    # hs8: [2(term), 16(chunk), 128, S] per batch
    hs8s = []
    for b in range(B):
        hsT = np.ascontiguousarray(hs[b].T)                  # [D, S]
        h1, h2 = _f8_split(hsT)
        hs8s.append(np.ascontiguousarray(
            np.stack([h1.reshape(NDC, 128, S), h2.reshape(NDC, 128, S)], 0)
        ).reshape(2 * D, S))

    in_maps = []
    for c in range(8):
        b, g = divmod(c, G)
        qnwg = np.ascontiguousarray(
            qnw_full[:, g * HPC * NOPE:(g + 1) * HPC * NOPE]).astype(f16)
        qrwg = np.ascontiguousarray(
            qrw_full[:, g * HPC * ROPE:(g + 1) * HPC * ROPE]).astype(f16)
        wukg = wuk_full[g * HPC * NOPE:(g + 1) * HPC * NOPE, :]
        wukTg = np.ascontiguousarray(np.concatenate(
            [wukg[hl * NOPE:(hl + 1) * NOPE, :].T for hl in range(HPC)],
            0)).astype(f16)
        wuvg = wuv_full[g * HPC * VD:(g + 1) * HPC * VD, :]
        wuv4g = np.ascontiguousarray(wuvg.T).astype(f16)
        owgv = np.ascontiguousarray(
            ow[g * HPC * VD:(g + 1) * HPC * VD, :]).astype(f16)
        in_maps.append({
            "hs8": hs8s[b],
            "qdw1": qdw1, "qdw2": qdw2,
            "kvdw1": kvdw1, "kvdw2": kvdw2,
            "krw1": krw1, "krw2": krw2,
            "qnw": qnwg, "qrw": qrwg, "wukT": wukTg, "wuv4": wuv4g,
            "owg": owgv,
            "cosd": cosT.astype(f16), "sind": sinT.astype(f16),
            "coskd": (cosT / P1_WSCALE).astype(f16),
            "sinkd": (sinT / P1_WSCALE).astype(f16),
            "maskd": maskv.astype(f16),
        })
    return in_maps


def kernel(**inputs):
    nc = _get_nc()
    in_maps = _host_prep(inputs)
    res = run_bass_kernel_spmd(nc, in_maps, core_ids=list(range(8)))
    out = np.zeros((B, S, D), np.float32)
    for c in range(8):
        out[c // G] += res.results[c]["outT"].astype(np.float32).T
    out += np.asarray(inputs["out_b"], np.float32)[None, None, :]
    return out
